# revision 1
# baseline (speedup 1.0000x reference)
"""GatedAttentionSublayer kernel for 8 Trainium2 NeuronCores.

Sharding: tensor-parallel over the H=16 attention heads (2 heads per
core). QKV / output-projection weights split cleanly per head; the
output projection partial sums are combined with an all-reduce (psum).
RMSNorm, gathers, gate and residual are computed replicated (cheap,
memory-regime). Runs SPMD on the 8 NeuronCores via PJRT.
"""

from functools import partial

import jax
import jax.numpy as jnp
import numpy as np

B, S, D = 2, 2048, 1024
H, DH = 16, 64
EPS = 1e-6
NDEV = 8
HPG = H // NDEV  # heads per core


@partial(
    jax.pmap,
    axis_name="i",
    in_axes=(None, None, None, None, None, 0, 0, 0, 0, 0, None),
)
def _run(x, mask, perm, inv_perm, gamma, wq, wk, wv, tau_l, wo_l, w_gate):
    b, s, d = x.shape
    rms = jnp.sqrt(jnp.mean(x * x, axis=-1, keepdims=True) + EPS)
    x_norm = (1.0 + gamma) * x / rms

    x_perm = jnp.take_along_axis(x_norm, perm[:, :, None], axis=1)
    pi = jnp.broadcast_to(perm[:, :, None], (b, s, s))
    pj = jnp.broadcast_to(perm[:, None, :], (b, s, s))
    mask_perm = jnp.take_along_axis(
        jnp.take_along_axis(mask, pi, axis=1), pj, axis=2
    )

    # local heads: wq/wk/wv are [D, HPG, DH]
    q = jnp.einsum("bsd,dhe->bhse", x_perm, wq)
    k = jnp.einsum("bsd,dhe->bhse", x_perm, wk)
    v = jnp.einsum("bsd,dhe->bhse", x_perm, wv)
    q = q / (jnp.linalg.norm(q, axis=-1, keepdims=True) + 1e-8)
    k = k / (jnp.linalg.norm(k, axis=-1, keepdims=True) + 1e-8)
    q = q * tau_l  # [HPG,1,1]

    logits = jnp.einsum("bhqd,bhkd->bhqk", q, k) / jnp.sqrt(jnp.float32(DH))
    logits = jnp.where(mask_perm[:, None, :, :], logits, jnp.finfo(logits.dtype).min)
    attn = jax.nn.softmax(logits, axis=-1)
    attn_out = jnp.einsum("bhqk,bhkd->bhqd", attn, v)

    # local slice of output projection, then all-reduce partials
    partial_o = jnp.einsum("bhqe,hed->bqd", attn_out, wo_l)  # wo_l [HPG, DH, D]
    attn_full = jax.lax.psum(partial_o, "i")

    attn_unperm = jnp.take_along_axis(attn_full, inv_perm[:, :, None], axis=1)
    gate = jax.nn.sigmoid(x_norm @ w_gate)
    return x + attn_unperm * gate


def kernel(x, mask, perm, gamma, w_qkv, tau, w_o, w_gate):
    x = np.asarray(x, dtype=np.float32)
    mask = np.asarray(mask)
    perm = np.asarray(perm, dtype=np.int32)
    gamma = np.asarray(gamma, dtype=np.float32)
    w_qkv = np.asarray(w_qkv, dtype=np.float32)
    tau = np.asarray(tau, dtype=np.float32)
    w_o = np.asarray(w_o, dtype=np.float32)
    w_gate = np.asarray(w_gate, dtype=np.float32)

    inv_perm = np.argsort(perm, axis=1).astype(np.int32)

    # split weights per head group: columns of w_qkv are [q(all H) | k | v],
    # head h owns cols h*DH:(h+1)*DH within each third.
    wq = w_qkv[:, 0 * D : 1 * D].reshape(D, NDEV, HPG, DH).transpose(1, 0, 2, 3)
    wk = w_qkv[:, 1 * D : 2 * D].reshape(D, NDEV, HPG, DH).transpose(1, 0, 2, 3)
    wv = w_qkv[:, 2 * D : 3 * D].reshape(D, NDEV, HPG, DH).transpose(1, 0, 2, 3)
    tau_l = tau.reshape(H)[: H].reshape(NDEV, HPG, 1, 1)
    # rows of w_o are the concat over heads of DH-dim blocks
    wo_l = w_o.reshape(H, DH, D).reshape(NDEV, HPG, DH, D)

    out = _run(
        x, mask, perm, inv_perm, gamma,
        np.ascontiguousarray(wq), np.ascontiguousarray(wk),
        np.ascontiguousarray(wv), tau_l, wo_l, w_gate,
    )
    return np.asarray(out[0], dtype=np.float32)
